# revision 15
# baseline (speedup 1.0000x reference)
# Trainium2 Bass kernel for nn_MultiCondLayer — Strassen level-1.
#   out = W'@x + b' (W' = sum_k W[k], b' = sum_k b[k]), mask applied on host.
#
# Strassen over 2x2 blocks (A = W' [2x2 of 512], B = x [2x2 of 512c x 2048n]):
#   M1=(A11+A22)(B11+B22) M2=(A21+A22)B11 M3=A11(B12-B22) M4=A22(B21-B11)
#   M5=(A11+A12)B22 M6=(A21-A11)(B11+B12) M7=(A12-A22)(B21+B22)
#   C11=M1+M4-M5+M7  C12=M3+M5  C21=M2+M4  C22=M1-M2+M3+M6
# 448 PE matmuls instead of 512 (-12.5% on the PE-bound critical path).
# A-combos are free on the host; B-combos are wide DVE/GpSimd tensor_tensor
# ops; C-recombination is folded into PSUM evictions via scalar_tensor_tensor
# (bias rides the stt scalar port). Host-simulated rel err 6.95e-3 (gate 2e-2).
#
# Work is split into n-halves h=0,1: block-0 cols h*1024.. and block-1 cols
# 2048+h*1024.. (x superchunks sc_h and sc_{2+h}). Per half: 7 Ms x 8 psum
# banks x 4 c-chunk matmuls. M order M2,M5,M4,M1,M7,M3,M6 so each C block
# completes (and stores) as early as possible.
#
# Engine split: stt evictions on DVE (GpSimd can't read PSUM); plain copy
# evictions on ACT (activation Identity); SBUF-only recombine tensor_tensors
# on GpSimd; B-combos round-robin DVE/GpSimd.

import numpy as np
import ml_dtypes

import concourse.bass as bass
import concourse.mybir as mybir
import concourse.tile as tile
from concourse import bacc
from concourse.bass_utils import run_bass_kernel_spmd

P = 128
B, C, N = 8, 1024, 4096
O = 1024
NT = 512
CO, OO = C // P, O // P
F32 = mybir.dt.float32
BF16 = mybir.dt.bfloat16
ADD = mybir.AluOpType.add
SUB = mybir.AluOpType.subtract
IDENT = mybir.ActivationFunctionType.Identity

N_CORES = 8
H = 512                  # block size (o and c)
NB = 2048                # n block size
NH = 512                 # n quarter width per block
NSUB = NH // NT          # 1


def build_module():
    nc = bacc.Bacc("TRN2", target_bir_lowering=False, debug=False,
                   num_devices=N_CORES)
    x = nc.dram_tensor("x", [C, N], BF16, kind="ExternalInput")
    # 7 pre-combined stationary operands, each [512c, 512o] (A_m.T)
    wts = nc.dram_tensor("wts", [7, H, H], BF16, kind="ExternalInput")
    bv = nc.dram_tensor("bv", [P, OO], F32, kind="ExternalInput")
    out = nc.dram_tensor("out", [O, N], BF16, kind="ExternalOutput")

    x_r = x.ap().rearrange("(c p) n -> p c n", p=P)        # [128, 8, N]
    w_r = wts.ap().rearrange("m (c p) o -> p m c o", p=P)  # [128, 7, 4, 512]
    out_r = out.ap().rearrange("(oo p) n -> p oo n", p=P)  # [128, 8, N]

    with tile.TileContext(nc) as tc:
        with (
            tc.tile_pool(name="consts", bufs=1) as consts,
            tc.tile_pool(name="xs", bufs=4) as xs,
            tc.tile_pool(name="cmbs", bufs=2) as cmbs,
            tc.tile_pool(name="ms", bufs=6) as ms,
            tc.tile_pool(name="tmps", bufs=8) as tmps,
            tc.tile_pool(name="outs", bufs=10) as outs,
            tc.tile_pool(name="ps", bufs=8, space="PSUM") as psp,
        ):
            wones = consts.tile([P, P], BF16)
            nc.vector.memset(wones[:], 0.125)
            xones = consts.tile([P, NT], BF16)
            nc.vector.memset(xones[:], 0.125)
            for i in range(4):
                wps = psp.tile([P, NT], F32, name=f"warm_{i}", tag="ps")
                nc.tensor.matmul(wps[:], wones[:], xones[:],
                                 start=True, stop=True)

            w_sb = consts.tile([P, 7, 4, H], BF16)
            bias_sb = consts.tile([P, OO], F32)
            # first matmul gates on M2's (m=1) c0 / o0:128 quarter
            nc.scalar.dma_start(w_sb[:, 1, 0, 0:P], w_r[:, 1, 0, 0:P])
            nc.scalar.dma_start(w_sb[:, 1, 0, P:H], w_r[:, 1, 0, P:H])
            nc.scalar.dma_start(w_sb[:, 1, 1:4, :], w_r[:, 1, 1:4, :])
            nc.scalar.dma_start(bias_sb[:], bv.ap())
            for m in (4, 3, 0, 6, 2, 5):   # M5, M4, M1, M7, M3, M6 order
                nc.scalar.dma_start(w_sb[:, m, :, :], w_r[:, m, :, :])

            # x superchunk tiles [128, 8c, 1024]; sc0/sc2 feed half 0,
            # sc1/sc3 feed half 1.
            x_sc = {}
            for h in range(4):
                sa = h * NH       # block-0 col offset
                sb = NB + h * NH  # block-1 col offset
                xa = xs.tile([P, CO, NH], BF16, name=f"xa_{h}", tag="xs")
                xb = xs.tile([P, CO, NH], BF16, name=f"xb_{h}", tag="xs")
                if h == 0:
                    # fine-grained first quarter (M2 consumes c0..c3 in order)
                    for c in range(CO):
                        nc.sync.dma_start(xa[:, c, :], x_r[:, c, 0:NH])
                    # M5 needs xb c4..7 first
                    nc.sync.dma_start(xb[:, 4:8, :], x_r[:, 4:8, sb:sb + NH])
                    nc.sync.dma_start(xb[:, 0:4, :], x_r[:, 0:4, sb:sb + NH])
                else:
                    nc.sync.dma_start(xa[:], x_r[:, :, sa:sa + NH])
                    nc.sync.dma_start(xb[:], x_r[:, :, sb:sb + NH])
                x_sc[h] = (xa, xb)

            for h in range(4):
                xa, xb = x_sc[h]
                n0a = h * NH          # block-0 col offset in out
                n0b = NB + h * NH     # block-1 col offset in out


                # SBUF M tiles we must retain (M6/M7 are consumed at evict)
                mt = {i: ms.tile([P, 4, NH], BF16, name=f"m{i}_{h}", tag="m")
                      for i in (0, 1, 2, 3, 4)}   # M1,M2,M3,M4,M5

                def combo(idx, s0, sl0, s1, sl1, op, eng):
                    cm = cmbs.tile([P, 4, NH], BF16,
                                   name=f"cmb{idx}_{h}", tag="cmb")
                    eng.tensor_tensor(cm[:], s0[:, sl0, :], s1[:, sl1, :],
                                      op=op)
                    return cm

                def mm(m, rhs_tile, rhs_base, pst):
                    # one M: 8 banks, cc-inner; returns dict of psum tiles
                    pss = {}
                    for oc in range(4):
                        for nsub in range(NSUB):
                            ps = psp.tile([P, NT], F32,
                                          name=f"ps_{h}_{pst}_{oc}_{nsub}",
                                          tag="ps")
                            pss[oc, nsub] = ps
                            for cc in range(4):
                                nc.tensor.matmul(
                                    ps[:],
                                    w_sb[:, m, cc, oc * P:(oc + 1) * P],
                                    rhs_tile[:, rhs_base + cc,
                                             nsub * NT:(nsub + 1) * NT],
                                    start=(cc == 0), stop=(cc == 3),
                                )
                    return pss

                def evict_copy(pss, dst):
                    # plain copy eviction on ACT (psum -> sbuf bf16)
                    for oc in range(4):
                        for nsub in range(NSUB):
                            nc.scalar.activation(
                                dst[:, oc, nsub * NT:(nsub + 1) * NT],
                                pss[oc, nsub][:], IDENT)

                # ---- M2 = A'(m=1) @ B11 ----
                ps2 = mm(1, xa, 0, "m2")
                evict_copy(ps2, mt[1])
                # ---- M5 = A'(m=4) @ B22 ----
                ps5 = mm(4, xb, 4, "m5")
                evict_copy(ps5, mt[4])
                # ---- M4 = A'(m=3) @ (B21-B11) ----
                cm4 = combo(4, xa, slice(4, 8), xa, slice(0, 4), SUB,
                            nc.vector)
                ps4 = mm(3, cm4, 0, "m4")
                # C21 = (M4 + bias1) + M2, done at M4's eviction (DVE stt);
                # M4 also copied for C11 (ACT).
                for oc in range(4):
                    otc21 = outs.tile([P, NSUB, NT], BF16,
                                      name=f"c21_{h}_{oc}", tag="ot")
                    for nsub in range(NSUB):
                        nc.vector.scalar_tensor_tensor(
                            otc21[:, nsub, :], ps4[oc, nsub][:],
                            bias_sb[:, 4 + oc:5 + oc],
                            mt[1][:, oc, nsub * NT:(nsub + 1) * NT],
                            op0=ADD, op1=ADD)
                        nc.scalar.activation(
                            mt[3][:, oc, nsub * NT:(nsub + 1) * NT],
                            ps4[oc, nsub][:], IDENT)
                    nc.sync.dma_start(
                        out_r[:, 4 + oc, n0a:n0a + NH], otc21[:])
                # ---- M1 = A'(m=0) @ (B11+B22) ----
                cm1 = combo(1, xa, slice(0, 4), xb, slice(4, 8), ADD,
                            nc.vector)
                ps1 = mm(0, cm1, 0, "m1")
                evict_copy(ps1, mt[0])
                # pre-combined partials, off the PE critical path:
                #   t1 = M1+M4 (GpSimd, slack until M7), t1b = t1-M5 (DVE)
                #   t3 = M1-M2 (GpSimd, slack until M6)
                t1b = {}
                t3 = {}
                for oc in range(4):
                    for nsub in range(NSUB):
                        sl = slice(nsub * NT, (nsub + 1) * NT)
                        t1 = tmps.tile([P, NT], BF16,
                                       name=f"t1_{h}_{oc}_{nsub}", tag="tmp")
                        nc.gpsimd.tensor_tensor(
                            t1[:], mt[0][:, oc, sl], mt[3][:, oc, sl], op=ADD)
                        tb = tmps.tile([P, NT], BF16,
                                       name=f"t1b_{h}_{oc}_{nsub}", tag="tmp")
                        nc.vector.tensor_tensor(
                            tb[:], t1[:], mt[4][:, oc, sl], op=SUB)
                        t1b[oc, nsub] = tb
                        tt3 = tmps.tile([P, NT], BF16,
                                        name=f"t3_{h}_{oc}_{nsub}", tag="tmp")
                        nc.gpsimd.tensor_tensor(
                            tt3[:], mt[0][:, oc, sl], mt[1][:, oc, sl],
                            op=SUB)
                        t3[oc, nsub] = tt3
                # ---- M7 = A'(m=6) @ (B21+B22) ----
                cm7 = combo(7, xa, slice(4, 8), xb, slice(4, 8), ADD,
                            nc.vector)
                ps7 = mm(6, cm7, 0, "m7")
                # C11 = (M7 + bias0) + (M1+M4-M5): completes at eviction
                for oc in range(4):
                    otc11 = outs.tile([P, NSUB, NT], BF16,
                                      name=f"c11_{h}_{oc}", tag="ot")
                    for nsub in range(NSUB):
                        nc.vector.scalar_tensor_tensor(
                            otc11[:, nsub, :], ps7[oc, nsub][:],
                            bias_sb[:, oc:oc + 1], t1b[oc, nsub][:],
                            op0=ADD, op1=ADD)
                    nc.sync.dma_start(
                        out_r[:, oc, n0a:n0a + NH], otc11[:])
                # ---- M3 = A'(m=2) @ (B12-B22) ----
                cm3 = combo(3, xb, slice(0, 4), xb, slice(4, 8), SUB,
                            nc.vector)
                ps3 = mm(2, cm3, 0, "m3")
                # C12 = (M3 + bias0) + M5 at eviction; M3 copied for C22;
                # t5 = (M1-M2) + M3 (DVE, slack until M6)
                t5 = {}
                for oc in range(4):
                    otc12 = outs.tile([P, NSUB, NT], BF16,
                                      name=f"c12_{h}_{oc}", tag="ot")
                    for nsub in range(NSUB):
                        sl = slice(nsub * NT, (nsub + 1) * NT)
                        nc.vector.scalar_tensor_tensor(
                            otc12[:, nsub, :], ps3[oc, nsub][:],
                            bias_sb[:, oc:oc + 1], mt[4][:, oc, sl],
                            op0=ADD, op1=ADD)
                        nc.scalar.activation(
                            mt[2][:, oc, sl], ps3[oc, nsub][:], IDENT)
                        tt5 = tmps.tile([P, NT], BF16,
                                        name=f"t5_{h}_{oc}_{nsub}", tag="tmp")
                        nc.vector.tensor_tensor(
                            tt5[:], t3[oc, nsub][:], mt[2][:, oc, sl],
                            op=ADD)
                        t5[oc, nsub] = tt5
                    nc.sync.dma_start(
                        out_r[:, oc, n0b:n0b + NH], otc12[:])
                # ---- M6 = A'(m=5) @ (B11+B12) ----
                cm6 = combo(6, xa, slice(0, 4), xb, slice(0, 4), ADD,
                            nc.vector)
                ps6 = mm(5, cm6, 0, "m6")
                # C22 = (M6 + bias1) + (M1-M2+M3): completes at eviction
                for oc in range(4):
                    otc22 = outs.tile([P, NSUB, NT], BF16,
                                      name=f"c22_{h}_{oc}", tag="ot")
                    for nsub in range(NSUB):
                        nc.vector.scalar_tensor_tensor(
                            otc22[:, nsub, :], ps6[oc, nsub][:],
                            bias_sb[:, 4 + oc:5 + oc], t5[oc, nsub][:],
                            op0=ADD, op1=ADD)
                    nc.sync.dma_start(
                        out_r[:, 4 + oc, n0b:n0b + NH], otc22[:])
    nc.compile()
    return nc


_NC_CACHE = None


def _get_module():
    global _NC_CACHE
    if _NC_CACHE is None:
        _NC_CACHE = build_module()
    return _NC_CACHE


def _make_in_maps(cond, x_mask, W, b):
    Wp = np.asarray(W, dtype=np.float32).sum(axis=0)      # [O, C]
    A11, A12 = Wp[:H, :H], Wp[:H, H:]
    A21, A22 = Wp[H:, :H], Wp[H:, H:]
    combos = [A11 + A22, A21 + A22, A11, A22, A11 + A12,
              A21 - A11, A12 - A22]
    wts = np.ascontiguousarray(
        np.stack([c.T for c in combos], axis=0).astype(ml_dtypes.bfloat16))
    bv = np.ascontiguousarray(
        np.asarray(b, dtype=np.float32).sum(axis=0).reshape(OO, P).T,
        dtype=np.float32)
    in_maps = []
    for core in range(N_CORES):
        in_maps.append({
            "x": np.ascontiguousarray(
                np.asarray(cond[core]).astype(ml_dtypes.bfloat16)),
            "wts": wts,
            "bv": bv,
        })
    return in_maps


def run(cond, x_mask, W, b, trace=False, trace_cores=None):
    nc = _get_module()
    in_maps = _make_in_maps(cond, x_mask, W, b)
    res = run_bass_kernel_spmd(
        nc, in_maps, core_ids=list(range(N_CORES)),
        trace=trace, trace_cores=trace_cores,
    )
    mask = np.asarray(x_mask, dtype=np.float32)
    out = np.stack(
        [np.asarray(res.results[i]["out"]).astype(np.float32)
         for i in range(N_CORES)], axis=0)
    out *= mask
    return out, res


def kernel(cond, x_mask, W, b):
    out, _ = run(cond, x_mask, W, b)
    return out


# revision 16
# speedup vs baseline: 1.1145x; 1.1145x over previous
# Trainium2 Bass kernel for nn_MultiCondLayer — Strassen level-1.
#   out = W'@x + b' (W' = sum_k W[k], b' = sum_k b[k]), mask applied on host.
#
# Strassen over 2x2 blocks (A = W' [2x2 of 512], B = x [2x2 of 512c x 2048n]):
#   M1=(A11+A22)(B11+B22) M2=(A21+A22)B11 M3=A11(B12-B22) M4=A22(B21-B11)
#   M5=(A11+A12)B22 M6=(A21-A11)(B11+B12) M7=(A12-A22)(B21+B22)
#   C11=M1+M4-M5+M7  C12=M3+M5  C21=M2+M4  C22=M1-M2+M3+M6
# 448 PE matmuls instead of 512 (-12.5% on the PE-bound critical path).
# A-combos are free on the host; B-combos are wide DVE/GpSimd tensor_tensor
# ops; C-recombination is folded into PSUM evictions via scalar_tensor_tensor
# (bias rides the stt scalar port). Host-simulated rel err 6.95e-3 (gate 2e-2).
#
# Work is split into n-halves h=0,1: block-0 cols h*1024.. and block-1 cols
# 2048+h*1024.. (x superchunks sc_h and sc_{2+h}). Per half: 7 Ms x 8 psum
# banks x 4 c-chunk matmuls. M order M2,M5,M4,M1,M7,M3,M6 so each C block
# completes (and stores) as early as possible.
#
# Engine split: stt evictions on DVE (GpSimd can't read PSUM); plain copy
# evictions on ACT (activation Identity); SBUF-only recombine tensor_tensors
# on GpSimd; B-combos round-robin DVE/GpSimd.

import numpy as np
import ml_dtypes

import concourse.bass as bass
import concourse.mybir as mybir
import concourse.tile as tile
from concourse import bacc
from concourse.bass_utils import run_bass_kernel_spmd

P = 128
B, C, N = 8, 1024, 4096
O = 1024
NT = 512
CO, OO = C // P, O // P
F32 = mybir.dt.float32
BF16 = mybir.dt.bfloat16
ADD = mybir.AluOpType.add
SUB = mybir.AluOpType.subtract
IDENT = mybir.ActivationFunctionType.Identity

N_CORES = 8
H = 512                  # block size (o and c)
NB = 2048                # n block size
NH = 512                 # n quarter width per block
NSUB = NH // NT          # 1


def build_module():
    nc = bacc.Bacc("TRN2", target_bir_lowering=False, debug=False,
                   num_devices=N_CORES)
    x = nc.dram_tensor("x", [C, N], BF16, kind="ExternalInput")
    # 7 pre-combined stationary operands, each [512c, 512o] (A_m.T)
    wts = nc.dram_tensor("wts", [7, H, H], BF16, kind="ExternalInput")
    bv = nc.dram_tensor("bv", [P, OO], F32, kind="ExternalInput")
    out = nc.dram_tensor("out", [O, N], BF16, kind="ExternalOutput")

    x_r = x.ap().rearrange("(c p) n -> p c n", p=P)        # [128, 8, N]
    w_r = wts.ap().rearrange("m (c p) o -> p m c o", p=P)  # [128, 7, 4, 512]
    out_r = out.ap().rearrange("(oo p) n -> p oo n", p=P)  # [128, 8, N]

    with tile.TileContext(nc) as tc:
        with (
            tc.tile_pool(name="consts", bufs=1) as consts,
            tc.tile_pool(name="xs", bufs=4) as xs,
            tc.tile_pool(name="cmbs", bufs=2) as cmbs,
            tc.tile_pool(name="ms", bufs=6) as ms,
            tc.tile_pool(name="tmps", bufs=8) as tmps,
            tc.tile_pool(name="outs", bufs=10) as outs,
            tc.tile_pool(name="ps", bufs=8, space="PSUM") as psp,
        ):
            wones = consts.tile([P, P], BF16)
            nc.vector.memset(wones[:], 0.125)
            xones = consts.tile([P, NT], BF16)
            nc.vector.memset(xones[:], 0.125)
            for i in range(4):
                wps = psp.tile([P, NT], F32, name=f"warm_{i}", tag="ps")
                nc.tensor.matmul(wps[:], wones[:], xones[:],
                                 start=True, stop=True)

            w_sb = consts.tile([P, 7, 4, H], BF16)
            bias_sb = consts.tile([P, OO], F32)
            # first matmul gates on M2's (m=1) c0 / o0:128 quarter
            nc.scalar.dma_start(w_sb[:, 1, 0, 0:P], w_r[:, 1, 0, 0:P])
            nc.scalar.dma_start(w_sb[:, 1, 0, P:H], w_r[:, 1, 0, P:H])
            nc.scalar.dma_start(w_sb[:, 1, 1:4, :], w_r[:, 1, 1:4, :])
            nc.scalar.dma_start(bias_sb[:], bv.ap())
            for m in (4, 3, 0, 6, 2, 5):   # M5, M4, M1, M7, M3, M6 order
                nc.scalar.dma_start(w_sb[:, m, :, :], w_r[:, m, :, :])

            # x superchunk tiles [128, 8c, 1024]; sc0/sc2 feed half 0,
            # sc1/sc3 feed half 1.
            x_sc = {}

            def load_x(h):
                # emitted lazily (inside quarter h-2) so pool-slot waits do
                # not head-of-line-block the stores on the Sync queue
                sa = h * NH       # block-0 col offset
                sb = NB + h * NH  # block-1 col offset
                xa = xs.tile([P, CO, NH], BF16, name=f"xa_{h}", tag="xs")
                xb = xs.tile([P, CO, NH], BF16, name=f"xb_{h}", tag="xs")
                if h == 0:
                    # fine-grained first quarter (M2 consumes c0..c3 in order)
                    for c in range(CO):
                        nc.sync.dma_start(xa[:, c, :], x_r[:, c, 0:NH])
                    # M5 needs xb c4..7 first
                    nc.sync.dma_start(xb[:, 4:8, :], x_r[:, 4:8, sb:sb + NH])
                    nc.sync.dma_start(xb[:, 0:4, :], x_r[:, 0:4, sb:sb + NH])
                else:
                    nc.sync.dma_start(xa[:], x_r[:, :, sa:sa + NH])
                    nc.sync.dma_start(xb[:], x_r[:, :, sb:sb + NH])
                x_sc[h] = (xa, xb)

            load_x(0)
            load_x(1)

            for h in range(4):
                xa, xb = x_sc[h]
                n0a = h * NH          # block-0 col offset in out
                n0b = NB + h * NH     # block-1 col offset in out


                # SBUF M tiles we must retain (M6/M7 are consumed at evict)
                mt = {i: ms.tile([P, 4, NH], BF16, name=f"m{i}_{h}", tag="m")
                      for i in (0, 1, 2, 3, 4)}   # M1,M2,M3,M4,M5

                def combo(idx, s0, sl0, s1, sl1, op, eng):
                    cm = cmbs.tile([P, 4, NH], BF16,
                                   name=f"cmb{idx}_{h}", tag="cmb")
                    eng.tensor_tensor(cm[:], s0[:, sl0, :], s1[:, sl1, :],
                                      op=op)
                    return cm

                def mm(m, rhs_tile, rhs_base, pst):
                    # one M: 8 banks, cc-inner; returns dict of psum tiles
                    pss = {}
                    for oc in range(4):
                        for nsub in range(NSUB):
                            ps = psp.tile([P, NT], F32,
                                          name=f"ps_{h}_{pst}_{oc}_{nsub}",
                                          tag="ps")
                            pss[oc, nsub] = ps
                            for cc in range(4):
                                nc.tensor.matmul(
                                    ps[:],
                                    w_sb[:, m, cc, oc * P:(oc + 1) * P],
                                    rhs_tile[:, rhs_base + cc,
                                             nsub * NT:(nsub + 1) * NT],
                                    start=(cc == 0), stop=(cc == 3),
                                )
                    return pss

                def evict_copy(pss, dst):
                    # plain copy eviction on ACT (psum -> sbuf bf16)
                    for oc in range(4):
                        for nsub in range(NSUB):
                            nc.scalar.activation(
                                dst[:, oc, nsub * NT:(nsub + 1) * NT],
                                pss[oc, nsub][:], IDENT)

                # ---- M2 = A'(m=1) @ B11 ----
                ps2 = mm(1, xa, 0, "m2")
                evict_copy(ps2, mt[1])
                if h + 2 < 4:
                    load_x(h + 2)
                # ---- M5 = A'(m=4) @ B22 ----
                ps5 = mm(4, xb, 4, "m5")
                evict_copy(ps5, mt[4])
                # ---- M4 = A'(m=3) @ (B21-B11) ----
                cm4 = combo(4, xa, slice(4, 8), xa, slice(0, 4), SUB,
                            nc.vector)
                ps4 = mm(3, cm4, 0, "m4")
                # C21 = (M4 + bias1) + M2, done at M4's eviction (DVE stt);
                # M4 also copied for C11 (ACT).
                for oc in range(4):
                    otc21 = outs.tile([P, NSUB, NT], BF16,
                                      name=f"c21_{h}_{oc}", tag="ot")
                    for nsub in range(NSUB):
                        nc.vector.scalar_tensor_tensor(
                            otc21[:, nsub, :], ps4[oc, nsub][:],
                            bias_sb[:, 4 + oc:5 + oc],
                            mt[1][:, oc, nsub * NT:(nsub + 1) * NT],
                            op0=ADD, op1=ADD)
                        nc.scalar.activation(
                            mt[3][:, oc, nsub * NT:(nsub + 1) * NT],
                            ps4[oc, nsub][:], IDENT)
                    nc.sync.dma_start(
                        out_r[:, 4 + oc, n0a:n0a + NH], otc21[:])
                # ---- M1 = A'(m=0) @ (B11+B22) ----
                cm1 = combo(1, xa, slice(0, 4), xb, slice(4, 8), ADD,
                            nc.vector)
                ps1 = mm(0, cm1, 0, "m1")
                evict_copy(ps1, mt[0])
                # pre-combined partials, off the PE critical path:
                #   t1 = M1+M4 (GpSimd, slack until M7), t1b = t1-M5 (DVE)
                #   t3 = M1-M2 (GpSimd, slack until M6)
                t1b = {}
                t3 = {}
                for oc in range(4):
                    for nsub in range(NSUB):
                        sl = slice(nsub * NT, (nsub + 1) * NT)
                        t1 = tmps.tile([P, NT], BF16,
                                       name=f"t1_{h}_{oc}_{nsub}", tag="tmp")
                        nc.gpsimd.tensor_tensor(
                            t1[:], mt[0][:, oc, sl], mt[3][:, oc, sl], op=ADD)
                        tb = tmps.tile([P, NT], BF16,
                                       name=f"t1b_{h}_{oc}_{nsub}", tag="tmp")
                        nc.gpsimd.tensor_tensor(
                            tb[:], t1[:], mt[4][:, oc, sl], op=SUB)
                        t1b[oc, nsub] = tb
                        tt3 = tmps.tile([P, NT], BF16,
                                        name=f"t3_{h}_{oc}_{nsub}", tag="tmp")
                        nc.gpsimd.tensor_tensor(
                            tt3[:], mt[0][:, oc, sl], mt[1][:, oc, sl],
                            op=SUB)
                        t3[oc, nsub] = tt3
                # ---- M7 = A'(m=6) @ (B21+B22) ----
                cm7 = combo(7, xa, slice(4, 8), xb, slice(4, 8), ADD,
                            nc.vector)
                ps7 = mm(6, cm7, 0, "m7")
                # C11 = (M7 + bias0) + (M1+M4-M5): completes at eviction
                for oc in range(4):
                    otc11 = outs.tile([P, NSUB, NT], BF16,
                                      name=f"c11_{h}_{oc}", tag="ot")
                    for nsub in range(NSUB):
                        nc.vector.scalar_tensor_tensor(
                            otc11[:, nsub, :], ps7[oc, nsub][:],
                            bias_sb[:, oc:oc + 1], t1b[oc, nsub][:],
                            op0=ADD, op1=ADD)
                    nc.sync.dma_start(
                        out_r[:, oc, n0a:n0a + NH], otc11[:])
                # ---- M3 = A'(m=2) @ (B12-B22) ----
                cm3 = combo(3, xb, slice(0, 4), xb, slice(4, 8), SUB,
                            nc.vector)
                ps3 = mm(2, cm3, 0, "m3")
                # C12 = (M3 + bias0) + M5 at eviction; M3 copied for C22;
                # t5 = (M1-M2) + M3 (DVE, slack until M6)
                t5 = {}
                for oc in range(4):
                    otc12 = outs.tile([P, NSUB, NT], BF16,
                                      name=f"c12_{h}_{oc}", tag="ot")
                    for nsub in range(NSUB):
                        sl = slice(nsub * NT, (nsub + 1) * NT)
                        nc.vector.scalar_tensor_tensor(
                            otc12[:, nsub, :], ps3[oc, nsub][:],
                            bias_sb[:, oc:oc + 1], mt[4][:, oc, sl],
                            op0=ADD, op1=ADD)
                        nc.scalar.activation(
                            mt[2][:, oc, sl], ps3[oc, nsub][:], IDENT)
                        tt5 = tmps.tile([P, NT], BF16,
                                        name=f"t5_{h}_{oc}_{nsub}", tag="tmp")
                        nc.gpsimd.tensor_tensor(
                            tt5[:], t3[oc, nsub][:], mt[2][:, oc, sl],
                            op=ADD)
                        t5[oc, nsub] = tt5
                    nc.sync.dma_start(
                        out_r[:, oc, n0b:n0b + NH], otc12[:])
                # ---- M6 = A'(m=5) @ (B11+B12) ----
                cm6 = combo(6, xa, slice(0, 4), xb, slice(0, 4), ADD,
                            nc.vector)
                ps6 = mm(5, cm6, 0, "m6")
                # C22 = (M6 + bias1) + (M1-M2+M3): completes at eviction
                for oc in range(4):
                    otc22 = outs.tile([P, NSUB, NT], BF16,
                                      name=f"c22_{h}_{oc}", tag="ot")
                    for nsub in range(NSUB):
                        nc.vector.scalar_tensor_tensor(
                            otc22[:, nsub, :], ps6[oc, nsub][:],
                            bias_sb[:, 4 + oc:5 + oc], t5[oc, nsub][:],
                            op0=ADD, op1=ADD)
                    nc.sync.dma_start(
                        out_r[:, 4 + oc, n0b:n0b + NH], otc22[:])
    nc.compile()
    return nc


_NC_CACHE = None


def _get_module():
    global _NC_CACHE
    if _NC_CACHE is None:
        _NC_CACHE = build_module()
    return _NC_CACHE


def _make_in_maps(cond, x_mask, W, b):
    Wp = np.asarray(W, dtype=np.float32).sum(axis=0)      # [O, C]
    A11, A12 = Wp[:H, :H], Wp[:H, H:]
    A21, A22 = Wp[H:, :H], Wp[H:, H:]
    combos = [A11 + A22, A21 + A22, A11, A22, A11 + A12,
              A21 - A11, A12 - A22]
    wts = np.ascontiguousarray(
        np.stack([c.T for c in combos], axis=0).astype(ml_dtypes.bfloat16))
    bv = np.ascontiguousarray(
        np.asarray(b, dtype=np.float32).sum(axis=0).reshape(OO, P).T,
        dtype=np.float32)
    in_maps = []
    for core in range(N_CORES):
        in_maps.append({
            "x": np.ascontiguousarray(
                np.asarray(cond[core]).astype(ml_dtypes.bfloat16)),
            "wts": wts,
            "bv": bv,
        })
    return in_maps


def run(cond, x_mask, W, b, trace=False, trace_cores=None):
    nc = _get_module()
    in_maps = _make_in_maps(cond, x_mask, W, b)
    res = run_bass_kernel_spmd(
        nc, in_maps, core_ids=list(range(N_CORES)),
        trace=trace, trace_cores=trace_cores,
    )
    mask = np.asarray(x_mask, dtype=np.float32)
    out = np.stack(
        [np.asarray(res.results[i]["out"]).astype(np.float32)
         for i in range(N_CORES)], axis=0)
    out *= mask
    return out, res


def kernel(cond, x_mask, W, b):
    out, _ = run(cond, x_mask, W, b)
    return out
